# revision 16
# baseline (speedup 1.0000x reference)
"""Trainium2 Bass kernel for nn_LNon_37460704756094 (embedding_lookup).

Math (reference):
    d   = (data - mean(data)) / std(data, ddof=1) * scalei
    s   = sigmoid(d); t = tanh(d)
    theta = interp(theta_lut, s * 119)   # theta_lut = linspace(-pi, pi, 120)
    velo  = interp(velo_lut, |t| * 119)  # velo_lut  = linspace(0, 3, 120)
    val = d * exp(velo * sin(theta)) + velo * cos(theta)
    out = (val - mean(val)) / std(val, ddof=1) * scaleo

Affine LUTs + sigmoid(d) = (1 + tanh(d/2))/2 give
    theta = th_mid + th_half * tanh(d/2);  velo = v_slope * |tanh(d)|
so all activations live in two table sets (tanh/exp/square | sin), with
sins batched per 8192-wide quarter: 2 table loads per quarter.

Per core (shard [128, 32768] f32), fp16 resident arrays x16/pb/qb:
  A:  stream x (32 x 1024, ring of 3, DMA alternating sync/gpsimd queues);
      scalar Copy+accum -> x16 (fp16) + sum(x); vector fused STT on x16
      gives sum(x^2) off the critical ring.
  AG1: 8-core AllGather of [128,2] partials (1 ring phase, warmed by a
      dummy AllGather at kernel start); strided reduce + ones-matmul
      broadcast; a = scalei/std, b = -mean*a.
  Mid, quarters of 8192 cols with the exp-set batch lagging one quarter:
      t=tanh((ax+b)/2)->pb | [exp-set work of q-1] | qb=sin(.+pi/2),
      pb=sin(.) in place | per 4096: T=tanh(ax+b); T=|T|; pb*=T; qb*=T;
      x16 = a*x16+b (u, in place).
      exp-set batch (1024 sub-chunks): e=exp(v_slope*p); e*=u;
      val = v_slope*q + e -> pb (accum sum); val^2 -> dead qb (accum).
  AG2, then D: out = a2*val + b2, ring of 3, DMA alternating queues.
"""

import math

import numpy as np

import concourse.bacc as bacc
import concourse.bass as bass
import concourse.mybir as mybir
import concourse.tile as tile
from concourse.bass_utils import run_bass_kernel_spmd

N_CORES = 8
P = 128
B_FULL, C, H, W = 32, 64, 128, 128
PER_CORE = B_FULL // N_CORES * C * H * W          # 4,194,304
FREE = PER_CORE // P                              # 32,768
N_TOTAL = B_FULL * C * H * W                      # 33,554,432

CA = 1024                                         # phase-A chunk
NA = FREE // CA                                   # 32
QW = 8192                                         # quarter width
NQ = FREE // QW                                   # 4
CT = 2048                                         # tanh-T chunk
CC = 1024                                         # exp/val chunk
NC = FREE // CC                                   # 32
CD = 1024                                         # store chunk
ND = FREE // CD                                   # 32

AF = mybir.ActivationFunctionType
ALU = mybir.AluOpType
AX = mybir.AxisListType
F32 = mybir.dt.float32
F16 = mybir.dt.float16

LAST_RESULT = None  # BassKernelResults of the most recent run (for test.py)

_KERNEL_CACHE = {}


def _build(consts, sim_mode=False):
    """consts = (th_mid, th_half, v_slope)."""
    th_mid, th_half, v_slope = consts
    halfpi = math.pi / 2.0

    nc = bacc.Bacc(None, num_devices=N_CORES)

    for cv in (th_mid, th_mid + halfpi):
        if (F32, cv) not in nc.const_aps.aps:
            t = nc.alloc_sbuf_tensor(f"const-f32-{cv}", [P, 1], F32)
            nc.gpsimd.memset(t.ap(), cv)
            nc.const_aps.aps[(F32, cv)] = t.ap()
    nc.all_engine_barrier()

    data_in = nc.dram_tensor("data", [P, FREE], F32, kind="ExternalInput")
    scal_in = nc.dram_tensor("scal", [P, 2], F32, kind="ExternalInput")
    out_dram = nc.dram_tensor("out", [P, FREE], F32, kind="ExternalOutput")

    groups = [list(range(N_CORES))]

    def all_gather(cc_in, cc_out):
        if sim_mode:
            for k in range(N_CORES):
                nc.gpsimd.dma_start(cc_out[k], cc_in[:])
        else:
            nc.gpsimd.collective_compute(
                "AllGather", ALU.bypass, replica_groups=groups,
                ins=[cc_in.opt()], outs=[cc_out.opt()],
            )

    with tile.TileContext(nc) as tc:
        with (
            tc.tile_pool(name="keep", bufs=1) as keep,
            tc.tile_pool(name="psum", bufs=1, space="PSUM") as psumpool,
            tc.tile_pool(name="dram", bufs=1, space="DRAM") as dram,
        ):
            # ------- persistent SBUF (192 KiB/partition + smalls) -------
            x16 = keep.tile([P, FREE], F16, name="x16", tag="x16")
            pb = keep.tile([P, FREE], F16, name="pb", tag="pb")
            qb = keep.tile([P, FREE], F16, name="qb", tag="qb")
            statA = keep.tile([P, 2 * NA], F32, name="statA", tag="statA")
            statC = keep.tile([P, 2 * NC], F32, name="statC", tag="statC")
            sm = keep.tile([P, 32], F32, name="sm", tag="sm")
            stA = keep.tile([P, 2], F32, name="stA", tag="stA")
            stB = keep.tile([P, 2], F32, name="stB", tag="stB")
            stAg = keep.tile([P, 2 * N_CORES], F32, name="stAg", tag="stAg")
            stBg = keep.tile([P, 2 * N_CORES], F32, name="stBg", tag="stBg")
            scal_all = keep.tile([P, 2], F32, name="scal_all", tag="scal_all")
            ones = keep.tile([P, P], F32, name="ones", tag="ones")
            psumA = psumpool.tile([P, 2], F32, name="psumA", tag="psumA")
            psumB = psumpool.tile([P, 2], F32, name="psumB", tag="psumB")

            cc_w_in = dram.tile([P, 2], F32, name="cc_w_in", tag="cc_w_in")
            cc_w_out = dram.tile([N_CORES, P, 2], F32, name="cc_w_out", tag="cc_w_out")
            cc_a_in = dram.tile([P, 2], F32, name="cc_a_in", tag="cc_a_in")
            cc_a_out = dram.tile([N_CORES, P, 2], F32, name="cc_a_out", tag="cc_a_out")
            cc_b_in = dram.tile([P, 2], F32, name="cc_b_in", tag="cc_b_in")
            cc_b_out = dram.tile([N_CORES, P, 2], F32, name="cc_b_out", tag="cc_b_out")

            # Two dummy AllGathers: the first two collectives on the cc
            # stream pay setup latency (~12us + ~30us measured); burn both
            # while phase A streams so the real ones run at steady state.
            all_gather(cc_w_in, cc_w_out)
            all_gather(cc_w_in, cc_w_out)

            nc.gpsimd.dma_start(scal_all[:], scal_in[:])
            nc.vector.memset(ones[:], 1.0)

            def gather_stats(st_in, cc_in, cc_out, st_g, st_out):
                """st_in [P,2] partials -> AllGather -> per-partition+core
                reduce -> matmul(ones) partition-reduce/broadcast -> st_out
                (a psum tile [P,2] holding the global sums)."""
                nc.gpsimd.dma_start(cc_in[:], st_in[:])
                all_gather(cc_in, cc_out)
                # gathered blob is [core, p, c]; land as [p, (core c)]
                nc.gpsimd.dma_start(
                    st_g[:].rearrange("p (k c) -> p k c", c=2),
                    cc_out[:].rearrange("k p c -> p k c"),
                )
                # sum over cores: [p, c, k] view, reduce innermost
                nc.vector.reduce_sum(
                    st_in[:], st_g[:].rearrange("p (k c) -> p c k", c=2),
                    axis=AX.X,
                )
                nc.tensor.matmul(st_out[:], ones[:], st_in[:])

            # ---------------- Phase A: load + convert + input stats ------
            # fp32 landing slots aliased into pb (unused until the mid
            # phase, whose first write trails AR1 anyway): ring of 6.
            NRING = 6
            xin = [
                pb[:, i * 2 * CA : (i + 1) * 2 * CA].bitcast(F32)
                for i in range(NRING)
            ]
            sq_dump = qb[:, 0:CA]  # discard target for the sumsq pass
            for j in range(NA):
                sl = slice(j * CA, (j + 1) * CA)
                xb = xin[j % NRING]
                eng = nc.sync if j % 2 == 0 else nc.gpsimd
                eng.dma_start(xb, data_in[:, sl])
                # fp32 -> fp16 convert + per-partition sum(x)
                nc.scalar.activation(
                    x16[:, sl], xb, AF.Copy,
                    accum_out=statA[:, j : j + 1],
                )
                # sum(x^2) from the fp16 copy (off the xin ring)
                nc.vector.scalar_tensor_tensor(
                    sq_dump, x16[:, sl], 1.0, x16[:, sl],
                    op0=ALU.mult, op1=ALU.mult,
                    accum_out=statA[:, NA + j : NA + j + 1],
                )

            nc.vector.reduce_sum(stA[:, 0:1], statA[:, 0:NA], axis=AX.X)
            nc.vector.reduce_sum(stA[:, 1:2], statA[:, NA : 2 * NA], axis=AX.X)

            gather_stats(stA, cc_a_in, cc_a_out, stAg, psumA)
            nc.vector.tensor_copy(sm[:, 0:2], psumA[:])

            # a = scalei / std, b = -mean * a   (std unbiased, ddof=1)
            nc.vector.tensor_scalar_mul(sm[:, 2:3], sm[:, 0:1], 1.0 / N_TOTAL)  # mean
            nc.vector.tensor_mul(sm[:, 3:4], sm[:, 0:1], sm[:, 2:3])            # S1*mean
            nc.vector.tensor_sub(sm[:, 4:5], sm[:, 1:2], sm[:, 3:4])
            nc.vector.tensor_scalar_mul(sm[:, 5:6], sm[:, 4:5], 1.0 / (N_TOTAL - 1))
            nc.scalar.activation(sm[:, 6:7], sm[:, 5:6], AF.Sqrt)               # std
            nc.vector.reciprocal(sm[:, 7:8], sm[:, 6:7])                        # 1/std
            nc.vector.tensor_mul(sm[:, 8:9], sm[:, 7:8], scal_all[:, 0:1])      # a
            nc.vector.tensor_mul(sm[:, 9:10], sm[:, 2:3], sm[:, 8:9])
            nc.vector.tensor_scalar_mul(sm[:, 10:11], sm[:, 9:10], -1.0)        # b
            nc.vector.tensor_scalar_mul(sm[:, 11:12], sm[:, 8:9], 0.5)          # a/2
            nc.vector.tensor_scalar_mul(sm[:, 12:13], sm[:, 10:11], 0.5)        # b/2
            a_ap = sm[:, 8:9]
            b_ap = sm[:, 10:11]
            ah_ap = sm[:, 11:12]
            bh_ap = sm[:, 12:13]

            # ---------- Mid: quarters, exp-set batch lagging one quarter --
            with tc.tile_pool(name="pm", bufs=1) as pm:
                Ts = [
                    pm.tile([P, CT], F16, name=f"T{i}", tag=f"T{i}")
                    for i in range(2)
                ]
                ee = [
                    pm.tile([P, CC], F16, name=f"ee{i}", tag=f"ee{i}")
                    for i in range(2)
                ]

                def emit_C(q):
                    """exp-set batch for quarter q: e, r, val, stats."""
                    for j in range(q * (QW // CC), (q + 1) * (QW // CC)):
                        sl = slice(j * CC, (j + 1) * CC)
                        e_ = ee[j % 2]
                        nc.scalar.activation(e_[:], pb[:, sl], AF.Exp, scale=v_slope)
                        # r = u * e  (in place over e)
                        nc.vector.tensor_mul(e_[:], x16[:, sl], e_[:])
                        # val = v_slope*q + r -> pb, accum sum(val)
                        nc.vector.scalar_tensor_tensor(
                            pb[:, sl], qb[:, sl], v_slope, e_[:],
                            op0=ALU.mult, op1=ALU.add,
                            accum_out=statC[:, j : j + 1],
                        )
                        # val^2 -> dead qb, accum sum(val^2)
                        nc.vector.scalar_tensor_tensor(
                            qb[:, sl], pb[:, sl], 1.0, pb[:, sl],
                            op0=ALU.mult, op1=ALU.mult,
                            accum_out=statC[:, NC + j : NC + j + 1],
                        )

                for qq in range(NQ):
                    qsl = slice(qq * QW, (qq + 1) * QW)
                    # t = tanh(d/2) -> pb   (tanh set, one 8192-wide inst)
                    nc.scalar.activation(
                        pb[:, qsl], x16[:, qsl], AF.Tanh, bias=bh_ap, scale=ah_ap
                    )
                    # exp-set batch of the previous quarter (same table set)
                    if qq > 0:
                        emit_C(qq - 1)
                    # sin set: cos first (reads t), then sin in place
                    nc.scalar.activation(
                        qb[:, qsl], pb[:, qsl], AF.Sin,
                        bias=th_mid + halfpi, scale=th_half,
                    )
                    nc.scalar.activation(
                        pb[:, qsl], pb[:, qsl], AF.Sin,
                        bias=th_mid, scale=th_half,
                    )
                    # tanh set: T, |T|, p, q, u per chunk (T_ double-buffered)
                    for h in range(QW // CT):
                        sl = slice(qq * QW + h * CT, qq * QW + (h + 1) * CT)
                        T_ = Ts[h % 2]
                        nc.scalar.activation(
                            T_[:], x16[:, sl], AF.Tanh, bias=b_ap, scale=a_ap
                        )
                        nc.vector.scalar_tensor_tensor(
                            T_[:], T_[:], -1.0, T_[:], op0=ALU.mult, op1=ALU.max
                        )
                        nc.vector.tensor_mul(pb[:, sl], pb[:, sl], T_[:])
                        nc.vector.tensor_mul(qb[:, sl], qb[:, sl], T_[:])
                        # u = a*x + b, in place over x16
                        nc.vector.tensor_scalar(
                            x16[:, sl], x16[:, sl], a_ap, b_ap,
                            op0=ALU.mult, op1=ALU.add,
                        )
                emit_C(NQ - 1)

                nc.vector.reduce_sum(stB[:, 0:1], statC[:, 0:NC], axis=AX.X)
                nc.vector.reduce_sum(stB[:, 1:2], statC[:, NC : 2 * NC], axis=AX.X)

            gather_stats(stB, cc_b_in, cc_b_out, stBg, psumB)
            nc.vector.tensor_copy(sm[:, 16:18], psumB[:])

            nc.vector.tensor_scalar_mul(sm[:, 18:19], sm[:, 16:17], 1.0 / N_TOTAL)
            nc.vector.tensor_mul(sm[:, 19:20], sm[:, 16:17], sm[:, 18:19])
            nc.vector.tensor_sub(sm[:, 20:21], sm[:, 17:18], sm[:, 19:20])
            nc.vector.tensor_scalar_mul(sm[:, 21:22], sm[:, 20:21], 1.0 / (N_TOTAL - 1))
            nc.scalar.activation(sm[:, 22:23], sm[:, 21:22], AF.Sqrt)
            nc.vector.reciprocal(sm[:, 23:24], sm[:, 22:23])
            nc.vector.tensor_mul(sm[:, 24:25], sm[:, 23:24], scal_all[:, 1:2])  # a2
            nc.vector.tensor_mul(sm[:, 25:26], sm[:, 18:19], sm[:, 24:25])
            nc.vector.tensor_scalar_mul(sm[:, 26:27], sm[:, 25:26], -1.0)       # b2
            a2_ap = sm[:, 24:25]
            b2_ap = sm[:, 26:27]

            # ---------------- Phase D: normalize + store -----------------
            # fp32 staging slots aliased into x16 (dead after the mid
            # phase): ring of 6.
            outs = [
                x16[:, i * 2 * CD : (i + 1) * 2 * CD].bitcast(F32)
                for i in range(NRING)
            ]
            for j in range(ND):
                sl = slice(j * CD, (j + 1) * CD)
                o_ = outs[j % NRING]
                nc.vector.tensor_scalar(
                    o_, pb[:, sl], a2_ap, b2_ap, op0=ALU.mult, op1=ALU.add
                )
                eng = nc.sync if j % 2 == 0 else nc.gpsimd
                eng.dma_start(out_dram[:, sl], o_)

    nc.finalize()
    return nc


def kernel(data, params, scalei, scaleo):
    global LAST_RESULT
    data = np.ascontiguousarray(np.asarray(data, dtype=np.float32))
    params = np.asarray(params, dtype=np.float32)

    # Affine-LUT coefficients from the actual params input.
    th_lut = params[0, 0]
    v_lut = params[1, 0]
    npts = th_lut.shape[0]
    th0 = float(th_lut[0])
    th_slope = float(th_lut[npts - 1]) - th0
    v0 = float(v_lut[0])
    v_slope = float(v_lut[npts - 1]) - v0
    assert abs(v0) < 1e-6, f"velocity LUT must start at 0 (got {v0})"

    # theta = th0 + th_slope*sigmoid(d) = th_mid + th_half*tanh(d/2)
    th_mid = th0 + 0.5 * th_slope
    th_half = 0.5 * th_slope

    consts = (th_mid, th_half, v_slope)
    nc = _KERNEL_CACHE.get(consts)
    if nc is None:
        nc = _build(consts)
        _KERNEL_CACHE[consts] = nc

    scal = np.tile(
        np.array(
            [[float(np.asarray(scalei).reshape(-1)[0]),
              float(np.asarray(scaleo).reshape(-1)[0])]],
            dtype=np.float32,
        ),
        (P, 1),
    )

    bpc = B_FULL // N_CORES
    in_maps = []
    for i in range(N_CORES):
        shard = np.ascontiguousarray(
            data[i * bpc : (i + 1) * bpc]
        ).reshape(P, FREE)
        in_maps.append({"data": shard, "scal": scal})

    res = run_bass_kernel_spmd(nc, in_maps, core_ids=list(range(N_CORES)))
    LAST_RESULT = res

    out = np.concatenate(
        [r["out"].reshape(bpc, C, H, W) for r in res.results], axis=0
    )
    return out


# revision 17
# speedup vs baseline: 1.0252x; 1.0252x over previous
"""Trainium2 Bass kernel for nn_LNon_37460704756094 (embedding_lookup).

Math (reference):
    d   = (data - mean(data)) / std(data, ddof=1) * scalei
    s   = sigmoid(d); t = tanh(d)
    theta = interp(theta_lut, s * 119)   # theta_lut = linspace(-pi, pi, 120)
    velo  = interp(velo_lut, |t| * 119)  # velo_lut  = linspace(0, 3, 120)
    val = d * exp(velo * sin(theta)) + velo * cos(theta)
    out = (val - mean(val)) / std(val, ddof=1) * scaleo

Affine LUTs + sigmoid(d) = (1 + tanh(d/2))/2 give
    theta = th_mid + th_half * tanh(d/2);  velo = v_slope * |tanh(d)|
so all activations live in two table sets (tanh/exp/square | sin), with
sins batched per 8192-wide quarter: 2 table loads per quarter.

Per core (shard [128, 32768] f32):
  A:  the whole fp32 shard streams into a full-size staging pool (xa,
      128 KiB/partition) in 32 chunks on two DMA queues with no buffer
      recycling; scalar Copy+accum converts to resident fp16 x16 and
      yields sum(x); a fused vector op yields sum(x^2).  The xa pool is
      then released; pb/qb (fp16) allocate into the freed zone, with
      framework overlap-deps ordering them behind AR1 anyway.
  AG1: 8-core AllGather of [128,2] partials (two dummy AllGathers at
      kernel start absorb the cc-stream warmup); strided reduce +
      ones-matmul broadcast; a = scalei/std, b = -mean*a.
  Mid, quarters of 8192 cols, exp-set batch lagging one quarter:
      t=tanh((ax+b)/2)->pb | [exp batch of q-1] | qb=sin(.+pi/2) then
      pb=sin(.) in place | per 2048: T=tanh(ax+b) (double-buffered);
      T=|T| in place; pb*=T; qb*=T; x16 = a*x16+b in place (u).
      exp batch (1024 sub-chunks): e=exp(v_slope*p); e*=u (r);
      val = v_slope*q + e -> pb (accum sum); val^2 -> dead qb (accum).
  AG2, then D: out = a2*val + b2 into a ring of 3 fp32 staging tiles
      (allocated in the freed mid-scratch zone), two DMA queues.

fp16 intermediates keep rel err ~1.6e-3 (gate 2e-2).
"""

import math

import numpy as np

import concourse.bacc as bacc
import concourse.bass as bass
import concourse.mybir as mybir
import concourse.tile as tile
from concourse.bass_utils import run_bass_kernel_spmd

N_CORES = 8
P = 128
B_FULL, C, H, W = 32, 64, 128, 128
PER_CORE = B_FULL // N_CORES * C * H * W          # 4,194,304
FREE = PER_CORE // P                              # 32,768
N_TOTAL = B_FULL * C * H * W                      # 33,554,432

CA = 1024                                         # phase-A chunk
NA = FREE // CA                                   # 32
QW = 8192                                         # quarter width
NQ = FREE // QW                                   # 4
CT = 2048                                         # tanh-T chunk
CC = 1024                                         # exp/val chunk
NC = FREE // CC                                   # 32
CD = 1024                                         # store chunk
ND = FREE // CD                                   # 32

AF = mybir.ActivationFunctionType
ALU = mybir.AluOpType
AX = mybir.AxisListType
F32 = mybir.dt.float32
F16 = mybir.dt.float16

LAST_RESULT = None  # BassKernelResults of the most recent run (for test.py)

_KERNEL_CACHE = {}


def _build(consts, sim_mode=False):
    """consts = (th_mid, th_half, v_slope)."""
    th_mid, th_half, v_slope = consts
    halfpi = math.pi / 2.0

    nc = bacc.Bacc(None, num_devices=N_CORES)

    for cv in (th_mid, th_mid + halfpi):
        if (F32, cv) not in nc.const_aps.aps:
            t = nc.alloc_sbuf_tensor(f"const-f32-{cv}", [P, 1], F32)
            nc.gpsimd.memset(t.ap(), cv)
            nc.const_aps.aps[(F32, cv)] = t.ap()
    nc.all_engine_barrier()

    data_in = nc.dram_tensor("data", [P, FREE], F32, kind="ExternalInput")
    scal_in = nc.dram_tensor("scal", [P, 2], F32, kind="ExternalInput")
    out_dram = nc.dram_tensor("out", [P, FREE], F32, kind="ExternalOutput")

    groups = [list(range(N_CORES))]

    def all_gather(cc_in, cc_out):
        if sim_mode:
            for k in range(N_CORES):
                nc.gpsimd.dma_start(cc_out[k], cc_in[:])
        else:
            nc.gpsimd.collective_compute(
                "AllGather", ALU.bypass, replica_groups=groups,
                ins=[cc_in.opt()], outs=[cc_out.opt()],
            )

    with tile.TileContext(nc) as tc:
        with (
            tc.tile_pool(name="keep", bufs=1) as keep,
            tc.tile_pool(name="psum", bufs=1, space="PSUM") as psumpool,
            tc.tile_pool(name="dram", bufs=1, space="DRAM") as dram,
        ):
            # ------- persistent SBUF -------
            x16 = keep.tile([P, FREE], F16, name="x16", tag="x16")
            statA = keep.tile([P, 2 * NA], F32, name="statA", tag="statA")
            statC = keep.tile([P, 2 * NC], F32, name="statC", tag="statC")
            sm = keep.tile([P, 32], F32, name="sm", tag="sm")
            stA = keep.tile([P, 2], F32, name="stA", tag="stA")
            stB = keep.tile([P, 2], F32, name="stB", tag="stB")
            stAg = keep.tile([P, 2 * N_CORES], F32, name="stAg", tag="stAg")
            stBg = keep.tile([P, 2 * N_CORES], F32, name="stBg", tag="stBg")
            scal_all = keep.tile([P, 2], F32, name="scal_all", tag="scal_all")
            ones = keep.tile([P, P], F32, name="ones", tag="ones")
            psumA = psumpool.tile([P, 2], F32, name="psumA", tag="psumA")
            psumB = psumpool.tile([P, 2], F32, name="psumB", tag="psumB")

            cc_w_in = dram.tile([P, 2], F32, name="cc_w_in", tag="cc_w_in")
            cc_w_out = dram.tile([N_CORES, P, 2], F32, name="cc_w_out", tag="cc_w_out")
            cc_a_in = dram.tile([P, 2], F32, name="cc_a_in", tag="cc_a_in")
            cc_a_out = dram.tile([N_CORES, P, 2], F32, name="cc_a_out", tag="cc_a_out")
            cc_b_in = dram.tile([P, 2], F32, name="cc_b_in", tag="cc_b_in")
            cc_b_out = dram.tile([N_CORES, P, 2], F32, name="cc_b_out", tag="cc_b_out")

            # Two dummy AllGathers: the first two collectives pay cc-stream
            # warmup (~12us + ~30us measured); burn both during phase A.
            all_gather(cc_w_in, cc_w_out)
            all_gather(cc_w_in, cc_w_out)

            nc.gpsimd.dma_start(scal_all[:], scal_in[:])
            nc.vector.memset(ones[:], 1.0)

            def gather_stats(st_in, cc_in, cc_out, st_g, st_out):
                """st_in [P,2] partials -> AllGather -> reduce over cores ->
                ones-matmul partition-reduce/broadcast into st_out (psum)."""
                nc.gpsimd.dma_start(cc_in[:], st_in[:])
                all_gather(cc_in, cc_out)
                nc.gpsimd.dma_start(
                    st_g[:].rearrange("p (k c) -> p k c", c=2),
                    cc_out[:].rearrange("k p c -> p k c"),
                )
                nc.vector.reduce_sum(
                    st_in[:], st_g[:].rearrange("p (k c) -> p c k", c=2),
                    axis=AX.X,
                )
                nc.tensor.matmul(st_out[:], ones[:], st_in[:])

            # ---------------- Phase A: load + convert + input stats ------
            # Full-size fp32 staging pool; released afterwards so pb/qb
            # can allocate into the same zone.
            with tc.tile_pool(name="pxa", bufs=1) as pxa:
                xa = pxa.tile([P, FREE], F32, name="xa", tag="xa")
                for j in range(NA):
                    sl = slice(j * CA, (j + 1) * CA)
                    eng = nc.sync if j % 2 == 0 else nc.gpsimd
                    eng.dma_start(xa[:, sl], data_in[:, sl])
                    # fp32 -> fp16 convert + per-partition sum(x)
                    nc.scalar.activation(
                        x16[:, sl], xa[:, sl], AF.Copy,
                        accum_out=statA[:, j : j + 1],
                    )
                    # sum(x^2), elementwise product dumped in place
                    nc.vector.scalar_tensor_tensor(
                        xa[:, sl], xa[:, sl], 1.0, xa[:, sl],
                        op0=ALU.mult, op1=ALU.mult,
                        accum_out=statA[:, NA + j : NA + j + 1],
                    )

                nc.vector.reduce_sum(stA[:, 0:1], statA[:, 0:NA], axis=AX.X)
                nc.vector.reduce_sum(stA[:, 1:2], statA[:, NA : 2 * NA], axis=AX.X)

            gather_stats(stA, cc_a_in, cc_a_out, stAg, psumA)
            nc.vector.tensor_copy(sm[:, 0:2], psumA[:])

            # a = scalei / std, b = -mean * a   (std unbiased, ddof=1)
            nc.vector.tensor_scalar_mul(sm[:, 2:3], sm[:, 0:1], 1.0 / N_TOTAL)  # mean
            nc.vector.tensor_mul(sm[:, 3:4], sm[:, 0:1], sm[:, 2:3])            # S1*mean
            nc.vector.tensor_sub(sm[:, 4:5], sm[:, 1:2], sm[:, 3:4])
            nc.vector.tensor_scalar_mul(sm[:, 5:6], sm[:, 4:5], 1.0 / (N_TOTAL - 1))
            nc.scalar.activation(sm[:, 6:7], sm[:, 5:6], AF.Sqrt)               # std
            nc.vector.reciprocal(sm[:, 7:8], sm[:, 6:7])                        # 1/std
            nc.vector.tensor_mul(sm[:, 8:9], sm[:, 7:8], scal_all[:, 0:1])      # a
            nc.vector.tensor_mul(sm[:, 9:10], sm[:, 2:3], sm[:, 8:9])
            nc.vector.tensor_scalar_mul(sm[:, 10:11], sm[:, 9:10], -1.0)        # b
            nc.vector.tensor_scalar_mul(sm[:, 11:12], sm[:, 8:9], 0.5)          # a/2
            nc.vector.tensor_scalar_mul(sm[:, 12:13], sm[:, 10:11], 0.5)        # b/2
            a_ap = sm[:, 8:9]
            b_ap = sm[:, 10:11]
            ah_ap = sm[:, 11:12]
            bh_ap = sm[:, 12:13]

            # pb/qb allocate into xa's released zone.
            with tc.tile_pool(name="pbq", bufs=1) as pbq:
                pb = pbq.tile([P, FREE], F16, name="pb", tag="pb")
                qb = pbq.tile([P, FREE], F16, name="qb", tag="qb")

                # ---- Mid: quarters, exp-set batch lagging one quarter ----
                with tc.tile_pool(name="pm", bufs=1) as pm:
                    Ts = [
                        pm.tile([P, CT], F16, name=f"T{i}", tag=f"T{i}")
                        for i in range(2)
                    ]
                    ee = [
                        pm.tile([P, CC], F16, name=f"ee{i}", tag=f"ee{i}")
                        for i in range(2)
                    ]

                    def emit_C(q):
                        """exp-set batch for quarter q: e, r, val, stats."""
                        for j in range(q * (QW // CC), (q + 1) * (QW // CC)):
                            sl = slice(j * CC, (j + 1) * CC)
                            e_ = ee[j % 2]
                            nc.scalar.activation(e_[:], pb[:, sl], AF.Exp, scale=v_slope)
                            # r = u * e  (in place over e)
                            nc.vector.tensor_mul(e_[:], x16[:, sl], e_[:])
                            # val = v_slope*q + r -> pb, accum sum(val)
                            nc.vector.scalar_tensor_tensor(
                                pb[:, sl], qb[:, sl], v_slope, e_[:],
                                op0=ALU.mult, op1=ALU.add,
                                accum_out=statC[:, j : j + 1],
                            )
                            # val^2 -> dead qb, accum sum(val^2)
                            nc.vector.scalar_tensor_tensor(
                                qb[:, sl], pb[:, sl], 1.0, pb[:, sl],
                                op0=ALU.mult, op1=ALU.mult,
                                accum_out=statC[:, NC + j : NC + j + 1],
                            )

                    for qq in range(NQ):
                        qsl = slice(qq * QW, (qq + 1) * QW)
                        # t = tanh(d/2) -> pb  (tanh set, one 8192-wide inst)
                        nc.scalar.activation(
                            pb[:, qsl], x16[:, qsl], AF.Tanh, bias=bh_ap, scale=ah_ap
                        )
                        # exp-set batch of the previous quarter (same set)
                        if qq > 0:
                            emit_C(qq - 1)
                        # sin set: cos first (reads t), then sin in place
                        nc.scalar.activation(
                            qb[:, qsl], pb[:, qsl], AF.Sin,
                            bias=th_mid + halfpi, scale=th_half,
                        )
                        nc.scalar.activation(
                            pb[:, qsl], pb[:, qsl], AF.Sin,
                            bias=th_mid, scale=th_half,
                        )
                        # tanh set: T, |T|, p, q, u per chunk
                        for h in range(QW // CT):
                            sl = slice(qq * QW + h * CT, qq * QW + (h + 1) * CT)
                            T_ = Ts[h % 2]
                            nc.scalar.activation(
                                T_[:], x16[:, sl], AF.Tanh, bias=b_ap, scale=a_ap
                            )
                            nc.vector.scalar_tensor_tensor(
                                T_[:], T_[:], -1.0, T_[:], op0=ALU.mult, op1=ALU.max
                            )
                            nc.vector.tensor_mul(pb[:, sl], pb[:, sl], T_[:])
                            nc.vector.tensor_mul(qb[:, sl], qb[:, sl], T_[:])
                            # u = a*x + b, in place over x16
                            nc.vector.tensor_scalar(
                                x16[:, sl], x16[:, sl], a_ap, b_ap,
                                op0=ALU.mult, op1=ALU.add,
                            )
                    emit_C(NQ - 1)

                    nc.vector.reduce_sum(stB[:, 0:1], statC[:, 0:NC], axis=AX.X)
                    nc.vector.reduce_sum(stB[:, 1:2], statC[:, NC : 2 * NC], axis=AX.X)

                gather_stats(stB, cc_b_in, cc_b_out, stBg, psumB)
                nc.vector.tensor_copy(sm[:, 16:18], psumB[:])

                nc.vector.tensor_scalar_mul(sm[:, 18:19], sm[:, 16:17], 1.0 / N_TOTAL)
                nc.vector.tensor_mul(sm[:, 19:20], sm[:, 16:17], sm[:, 18:19])
                nc.vector.tensor_sub(sm[:, 20:21], sm[:, 17:18], sm[:, 19:20])
                nc.vector.tensor_scalar_mul(sm[:, 21:22], sm[:, 20:21], 1.0 / (N_TOTAL - 1))
                nc.scalar.activation(sm[:, 22:23], sm[:, 21:22], AF.Sqrt)
                nc.vector.reciprocal(sm[:, 23:24], sm[:, 22:23])
                nc.vector.tensor_mul(sm[:, 24:25], sm[:, 23:24], scal_all[:, 1:2])  # a2
                nc.vector.tensor_mul(sm[:, 25:26], sm[:, 18:19], sm[:, 24:25])
                nc.vector.tensor_scalar_mul(sm[:, 26:27], sm[:, 25:26], -1.0)       # b2
                a2_ap = sm[:, 24:25]
                b2_ap = sm[:, 26:27]

                # ---------------- Phase D: normalize + store -------------
                with tc.tile_pool(name="pd", bufs=1) as pd:
                    outs = [
                        pd.tile([P, CD], F32, name=f"o{i}", tag=f"o{i}")
                        for i in range(3)
                    ]
                    for j in range(ND):
                        sl = slice(j * CD, (j + 1) * CD)
                        o_ = outs[j % 3]
                        nc.vector.tensor_scalar(
                            o_[:], pb[:, sl], a2_ap, b2_ap, op0=ALU.mult, op1=ALU.add
                        )
                        eng = nc.sync if j % 2 == 0 else nc.gpsimd
                        eng.dma_start(out_dram[:, sl], o_[:])

    nc.finalize()
    return nc


def kernel(data, params, scalei, scaleo):
    global LAST_RESULT
    data = np.ascontiguousarray(np.asarray(data, dtype=np.float32))
    params = np.asarray(params, dtype=np.float32)

    # Affine-LUT coefficients from the actual params input.
    th_lut = params[0, 0]
    v_lut = params[1, 0]
    npts = th_lut.shape[0]
    th0 = float(th_lut[0])
    th_slope = float(th_lut[npts - 1]) - th0
    v0 = float(v_lut[0])
    v_slope = float(v_lut[npts - 1]) - v0
    assert abs(v0) < 1e-6, f"velocity LUT must start at 0 (got {v0})"

    # theta = th0 + th_slope*sigmoid(d) = th_mid + th_half*tanh(d/2)
    th_mid = th0 + 0.5 * th_slope
    th_half = 0.5 * th_slope

    consts = (th_mid, th_half, v_slope)
    nc = _KERNEL_CACHE.get(consts)
    if nc is None:
        nc = _build(consts)
        _KERNEL_CACHE[consts] = nc

    scal = np.tile(
        np.array(
            [[float(np.asarray(scalei).reshape(-1)[0]),
              float(np.asarray(scaleo).reshape(-1)[0])]],
            dtype=np.float32,
        ),
        (P, 1),
    )

    bpc = B_FULL // N_CORES
    in_maps = []
    for i in range(N_CORES):
        shard = np.ascontiguousarray(
            data[i * bpc : (i + 1) * bpc]
        ).reshape(P, FREE)
        in_maps.append({"data": shard, "scal": scal})

    res = run_bass_kernel_spmd(nc, in_maps, core_ids=list(range(N_CORES)))
    LAST_RESULT = res

    out = np.concatenate(
        [r["out"].reshape(bpc, C, H, W) for r in res.results], axis=0
    )
    return out
